# revision 23
# baseline (speedup 1.0000x reference)
"""Contextual LSTM cell on 8 Trainium2 NeuronCores.

Strategy:
  - Shard the batch dim (B=65536) across 8 cores (8192 each), replicate weights.
  - All 15 gate matmuls fused into ONE (1024 x 832) @ (832 x B) matmul:
        rows:  [gate_i | gate_f | gate_c | gate_o]      (4 x 256)
        cols:  [x (256) | h (256) | c (256) | topic (64)]
    with -w_ho folded in and the (gate_c, c) block identically zero (skipped).
  - Matmul in fp16 (1 cycle/row on PE vs 4 for fp32), accumulated fp32 in PSUM.
    x/h/topic + weights are cast to fp16 on the host; c is shipped fp32 (needed
    exactly for cc = cf*c + ...) and cast to fp16 on-device for the matmul.
  - Per-gate bias is fused into the ScalarE activation (sigmoid/tanh) that
    reads the PSUM bank directly; cc/ch elementwise runs fp32 on VectorE.
"""

import os
import numpy as np

import concourse.bass as bass
import concourse.bacc as bacc
import concourse.mybir as mybir
from concourse.tile import TileContext
from concourse.bass_utils import run_bass_kernel_spmd

I, H, T, B = 256, 256, 64, 65536
NCORES = 8
BS = B // NCORES          # 8192 batch columns per core
NT = 512                  # columns per chunk (one PSUM bank of fp32)
NCHUNK = BS // NT         # 16
KB = 7                    # k-blocks: x:2, h:2, c:2, topic:1(64 rows)
MB = 8                    # m-blocks: 4 gates x 2

FP16 = mybir.dt.float16
FP32 = mybir.dt.float32
SIG = mybir.ActivationFunctionType.Sigmoid
TANH = mybir.ActivationFunctionType.Tanh

_PROGRAM = None
_LAST_RESULTS = None  # for test harness introspection


def _build_program(repeat=1):
    # Bacc (not Bass): its compile() pass splits multi-semaphore waits into
    # InstEventSemaphore preludes — walrus rejects >1 sync wait per
    # instruction otherwise.
    nc = bacc.Bacc()

    xh = nc.declare_dram_parameter("xh", [2 * H, BS], FP16, isOutput=False)
    c_in = nc.declare_dram_parameter("c", [H, BS], FP32, isOutput=False)
    topic = nc.declare_dram_parameter("topic", [T, BS], FP16, isOutput=False)
    wt = nc.declare_dram_parameter("wt", [128, KB * 1024], FP16, isOutput=False)
    biases = nc.declare_dram_parameter("biases", [128, MB], FP32, isOutput=False)
    ch_out = nc.declare_dram_parameter("ch", [H, BS], FP32, isOutput=True)
    cc_out = nc.declare_dram_parameter("cc", [H, BS], FP32, isOutput=True)

    # 15 chunks of 512 columns + the last 512 split in two — the half-size
    # tail chunks shorten the post-matmul drain of the pipeline.
    chunks = [(i * NT, NT) for i in range(NCHUNK - 1)]
    chunks += [((NCHUNK - 1) * NT, NT // 2), ((NCHUNK - 1) * NT + NT // 2, NT // 2)]

    with TileContext(nc) as tc:
        with (
            tc.tile_pool(name="const", bufs=1) as constp,
            tc.tile_pool(name="zin", bufs=4) as zp,
            tc.tile_pool(name="gates", bufs=2) as gp,
            tc.tile_pool(name="psum", bufs=1, space="PSUM") as pp,
        ):
            wsb = constp.tile([128, KB * 1024], FP16, tag="w", name="wsb")
            bsb = constp.tile([128, MB], FP32, tag="b", name="bsb")
            # k0 weights first: the very first matmul needs only this block.
            # Remaining weight blocks + biases are interleaved between the
            # first chunk's input DMAs below (8 HWDGE queues run them in
            # parallel) so the PE can start ~2us after kernel entry.
            nc.sync.dma_start(out=wsb[:, 0:1024], in_=wt[:, 0:1024])
            pending_w = list(range(1, KB)) + [None]  # None -> bias DMA

            def weight_dma():
                if not pending_w:
                    return
                k = pending_w.pop(0)
                if k is None:
                    nc.sync.dma_start(out=bsb[:], in_=biases[:])
                else:
                    nc.sync.dma_start(out=wsb[:, k * 1024:(k + 1) * 1024],
                                      in_=wt[:, k * 1024:(k + 1) * 1024])

            # PE warm-up: ~2.5us of tiny matmuls hidden under the initial DMA
            # fill releases the HAM clock gate (cold PE runs at 1.2 GHz for
            # its first ~3.4us of activity) before the real stream begins.
            wz = constp.tile([128, 64], FP16, tag="wz", name="wz")
            nc.vector.memset(wz[:], 0.0)
            pdum = pp.tile([128, NT], FP32, tag="ps0", name="pdum")
            for _ in range(28):
                nc.tensor.matmul(pdum[0:64, 0:64], wz[:, 0:64], wz[:, 0:64],
                                 start=True, stop=True)

            for rn in range(repeat * len(chunks)):
                r, n = divmod(rn, len(chunks))
                c0, nt = chunks[n]
                c1 = c0 + nt

                # ---- load inputs for this column chunk ----
                z = []
                for j in range(4):  # x0 x1 h0 h1, fp16 direct from DRAM
                    zj = zp.tile([128, nt], FP16, tag=f"z{j}", name=f"z{j}_{n}")
                    nc.sync.dma_start(out=zj[:], in_=xh[j * 128:(j + 1) * 128, c0:c1])
                    if rn == 0:
                        weight_dma()
                    z.append(zj)
                cf32 = []
                for j in range(2):  # c fp32 (kept for elementwise) + fp16 cast
                    cj = zp.tile([128, nt], FP32, tag=f"c{j}", name=f"c{j}_{n}")
                    nc.sync.dma_start(out=cj[:], in_=c_in[j * 128:(j + 1) * 128, c0:c1])
                    if rn == 0:
                        weight_dma()
                    c16 = zp.tile([128, nt], FP16, tag=f"c16_{j}", name=f"c16_{j}_{n}")
                    nc.vector.tensor_copy(out=c16[:], in_=cj[:])
                    cf32.append(cj)
                    z.append(c16)
                # topic duplicated into partitions 64-127 so the two topic
                # matmuls of an m-pair can run concurrently via row packing
                tp = zp.tile([128, nt], FP16, tag="tp", name=f"tp_{n}")
                nc.sync.dma_start(out=tp[0:T, :], in_=topic[:, c0:c1])
                if rn == 0:
                    weight_dma()
                nc.sync.dma_start(out=tp[T:128, :], in_=topic[:, c0:c1])
                if rn == 0:
                    weight_dma()

                # ---- the fused gate matmul ----
                # Per m-pair: all K=128 blocks of both m's first, then the two
                # K=64 topic matmuls back-to-back on distinct row-groups
                # ((0,0) and (64,0)) — they execute concurrently in the PE
                # array, halving the topic block's cost.
                ps = [None] * MB
                for m0 in range(0, MB, 2):
                    for m in (m0, m0 + 1):
                        pst = pp.tile([128, nt], FP32, tag=f"ps{m}", name=f"ps{m}_{n}")
                        # gate_c (m 4,5) has no c-term: skip the zero blocks
                        ks = [k for k in range(6) if not (m in (4, 5) and k in (4, 5))]
                        for i, k in enumerate(ks):
                            lhsT = wsb[:, k * 1024 + m * 128: k * 1024 + (m + 1) * 128]
                            nc.tensor.matmul(
                                pst[:], lhsT, z[k][:],
                                start=(i == 0), stop=False,
                            )
                        ps[m] = pst
                    for m in (m0, m0 + 1):
                        p0 = T * (m & 1)
                        lhsT = wsb[p0:p0 + T, 6 * 1024 + m * 128: 6 * 1024 + (m + 1) * 128]
                        nc.tensor.matmul(
                            ps[m][:], lhsT, tp[p0:p0 + T, :],
                            start=False, stop=True,
                            tile_position=(p0, 0),
                        )

                # ---- gate activations (bias fused, reads PSUM) ----
                def act(m, fn, nm):
                    t = gp.tile([128, nt], FP32, tag=nm, name=f"{nm}_{n}")
                    nc.scalar.activation(out=t[:], in_=ps[m][:], func=fn,
                                         bias=bsb[:, m:m + 1])
                    return t

                for half in range(2):
                    ci = act(0 + half, SIG, f"ci{half}")
                    cf = act(2 + half, SIG, f"cf{half}")
                    tg = act(4 + half, TANH, f"tg{half}")
                    co = act(6 + half, SIG, f"co{half}")

                    t1 = gp.tile([128, nt], FP32, tag=f"t1{half}", name=f"t1{half}_{n}")
                    nc.vector.tensor_mul(t1[:], ci[:], tg[:])
                    t2 = gp.tile([128, nt], FP32, tag=f"t2{half}", name=f"t2{half}_{n}")
                    nc.vector.tensor_mul(t2[:], cf[:], cf32[half][:])
                    cct = gp.tile([128, nt], FP32, tag=f"cc{half}", name=f"cc{half}_{n}")
                    nc.vector.tensor_add(cct[:], t1[:], t2[:])
                    tcc = gp.tile([128, nt], FP32, tag=f"tcc{half}", name=f"tcc{half}_{n}")
                    nc.scalar.activation(out=tcc[:], in_=cct[:], func=TANH)
                    cht = gp.tile([128, nt], FP32, tag=f"chh{half}", name=f"chh{half}_{n}")
                    nc.vector.tensor_mul(cht[:], co[:], tcc[:])

                    r0, r1 = half * 128, (half + 1) * 128
                    nc.sync.dma_start(out=cc_out[r0:r1, c0:c1], in_=cct[:])
                    nc.sync.dma_start(out=ch_out[r0:r1, c0:c1], in_=cht[:])

    nc.finalize()
    return nc


def _prep_weights(inp):
    """Assemble the fused (1024, 832) weight and return lhsT blocks + biases."""
    Wf = np.zeros((1024, 832), np.float32)

    def put(g, blocks):
        r = g * 256
        for j, wb in enumerate(blocks):
            if wb is None:
                continue
            col = j * 256
            Wf[r:r + 256, col:col + wb.shape[1]] = wb

    put(0, [inp["w_ii"], inp["w_hi"], inp["w_ci"], inp["w_bi"]])
    put(1, [inp["w_if"], inp["w_hf"], inp["w_cf"], inp["w_bf"]])
    put(2, [inp["w_ic"], inp["w_hc"], None, inp["w_bc"]])
    put(3, [inp["w_io"], -inp["w_ho"], inp["w_co"], inp["w_bo"]])

    wT = np.zeros((KB * 128, 1024), np.float32)
    wT[:832] = Wf.T
    # duplicate topic weight rows into partitions 64-127 of the k=6 block
    # (row-packed topic matmuls read them at base_partition 64)
    wT[832:896] = wT[768:832]
    # (7,128,1024) -> (128, 7*1024) with block-k contiguous in the free dim
    wt_host = np.ascontiguousarray(
        wT.reshape(KB, 128, 1024).transpose(1, 0, 2).reshape(128, KB * 1024)
    ).astype(np.float16)

    bias_all = np.concatenate(
        [inp["bias_i"], inp["bias_f"], inp["bias_c"], inp["bias_o"]], axis=0
    ).reshape(MB, 128)
    bias_host = np.ascontiguousarray(bias_all.T).astype(np.float32)
    return wt_host, bias_host


def kernel(**inputs):
    global _PROGRAM, _LAST_RESULTS
    if _PROGRAM is None:
        _PROGRAM = _build_program()
    nc = _PROGRAM

    inp = {k: np.asarray(v, dtype=np.float32) for k, v in inputs.items()}
    wt_host, bias_host = _prep_weights(inp)

    xh16 = np.concatenate([inp["x"], inp["h"]], axis=0).astype(np.float16)
    t16 = inp["topic"].astype(np.float16)
    c32 = inp["c"]

    in_maps = []
    for i in range(NCORES):
        sl = slice(i * BS, (i + 1) * BS)
        in_maps.append({
            "xh": np.ascontiguousarray(xh16[:, sl]),
            "c": np.ascontiguousarray(c32[:, sl]),
            "topic": np.ascontiguousarray(t16[:, sl]),
            "wt": wt_host,
            "biases": bias_host,
        })

    res = run_bass_kernel_spmd(
        nc, in_maps, list(range(NCORES)),
        trace=bool(os.environ.get("KERNEL_TRACE")),
    )
    _LAST_RESULTS = res

    ch = np.concatenate([res.results[i]["ch"] for i in range(NCORES)], axis=1)
    cc = np.concatenate([res.results[i]["cc"] for i in range(NCORES)], axis=1)
    return np.stack([ch, cc], axis=0)
